# revision 24
# baseline (speedup 1.0000x reference)
"""Trainium2 Bass kernel for GAT-style attention score computation.

Math (see reference):
    s_src = X @ a[:F];  s_dst = X @ a[F:]
    e[i, j] = leaky_relu(s_src[i] + s_dst[j], alpha=0.2)

Sharding over 8 NeuronCores: row-shard X (1024 rows/core). Each core
computes its local s_src/s_dst slices, AllGathers s_dst (8192 floats),
and emits its [1024, 8192] row block of e.

Per-core dataflow (v3, "pipe"):
  - Row mapping: local row c*128 + p lives at partition p, sub-row c.
    Each [128, 8192] output store is then ONE contiguous 4 MB HBM block
    (measured 346 GB/s vs 317 GB/s for the interleaved mapping).
  - X shard loaded as one SBUF tile [128, 2048] via a 3-D AP.
  - a_src/a_dst broadcast to [128, 256] via a ones[1,128] matmul.
  - s_dst/s_src matvecs split DVE/GpSimd (4 sub-rows each).
  - s_dst [128, 8] -> PE transpose (identity built on-chip with
    affine_select) -> [8, 128] -> DRAM in local row order -> AllGather.
  - Gathered s_dst [1, 8192] -> broadcast + activation pipelined in
    1024-column chunks: per chunk 2 ones-matmuls + copies -> dbk
    [128, 1024], then 8 ScalarE activations Lrelu(dbk + s_src[:, t])
    each DMA'd to a contiguous 512 KB block of the output.
    First output DMA starts ~5 us after the gather lands; the bcast
    build for chunk k+1 overlaps chunk k's stores.

The kernel is output-write bound: 32 MB/core of f32 at ~358 GB/s HBM
(~97 us floor); ScalarE activation total is ~73 us and hides under it.

`repeat` (bench-only): wraps the body in a For_i hardware loop with the
collective hoisted to a prologue (collectives inside a loop desync the
mesh), so device time can be measured by slope against the repeat count
(the axon per-call dispatch overhead is ~68 ms and swamps a single
execution).
"""

import numpy as np

N = 8192
F = 256
NCORES = 8
ROWS = N // NCORES          # 1024 rows per core
P = 128                     # partitions
C = ROWS // P               # 8 sub-rows per partition
ALPHA = 0.2
NB = 512                    # PSUM-bank-sized bcast matmul width
NB2 = 1024                  # column chunk width of the act/store pipeline

_CACHE = {}


def _build(repeat=1):
    import concourse.bacc as bacc
    import concourse.bass as bass
    import concourse.tile as tile
    from concourse import mybir
    from contextlib import nullcontext

    fp32 = mybir.dt.float32

    nc = bacc.Bacc(
        "TRN2",
        target_bir_lowering=False,
        debug=False,
        num_devices=NCORES,
    )

    x_dram = nc.dram_tensor("feature_matrix", [ROWS, F], fp32, kind="ExternalInput")
    av_dram = nc.dram_tensor("attention_vector", [2 * F, 1], fp32, kind="ExternalInput")
    out_dram = nc.dram_tensor("out", [ROWS, N], fp32, kind="ExternalOutput")

    with tile.TileContext(nc) as tc:
        with (
            tc.tile_pool(name="const", bufs=1) as const_pool,
            tc.tile_pool(name="work", bufs=2) as work_pool,
            tc.tile_pool(name="dbc", bufs=2) as dbc_pool,
            tc.tile_pool(name="outp", bufs=8) as out_pool,
            tc.tile_pool(name="psum", bufs=4, space=bass.MemorySpace.PSUM) as psum_pool,
            tc.tile_pool(name="psum1", bufs=1, space=bass.MemorySpace.PSUM) as ps1_pool,
            tc.tile_pool(name="dram", bufs=1, space="DRAM") as dram_pool,
        ):
            cc_in = dram_pool.tile([P, C], fp32, tag="cc_in")
            cc_out = dram_pool.tile([2 * C, N // (2 * C)], fp32, tag="cc_out")

            def collective():
                nc.gpsimd.collective_compute(
                    "AllGather",
                    mybir.AluOpType.bypass,
                    replica_groups=[list(range(NCORES))],
                    ins=[cc_in[:].opt()],
                    outs=[cc_out[:].opt()],
                )

            # ---- constants shared by prologue and loop ----
            av_sb = const_pool.tile([1, 2 * F], fp32, tag="av_sb")
            nc.sync.dma_start(av_sb[:], av_dram.ap().rearrange("f one -> one f"))
            ones_sb = const_pool.tile([1, P], fp32, tag="ones_sb")
            nc.vector.memset(ones_sb[:], 1.0)
            ident = const_pool.tile([P, P], fp32, tag="ident")
            nc.gpsimd.memset(ident[:], 1.0)
            nc.gpsimd.affine_select(
                ident[:], ident[:], pattern=[[1, P]],
                compare_op=mybir.AluOpType.is_equal, fill=0.0,
                base=0, channel_multiplier=-1,
            )
            a_ps = ps1_pool.tile([P, 2 * F], fp32, tag="a_ps")
            nc.tensor.matmul(a_ps[:], ones_sb[:], av_sb[:], start=True, stop=True)
            ab_sb = const_pool.tile([P, 2 * F], fp32, tag="ab_sb")
            nc.vector.tensor_copy(ab_sb[:], a_ps[:])

            def load_x(tag):
                x_sb = const_pool.tile([P, C * F], fp32, tag=tag)
                nc.sync.dma_start(
                    x_sb[:].rearrange("p (c f) -> p c f", c=C),
                    x_dram.ap().rearrange("(c p) f -> p c f", c=C),
                )
                return x_sb

            def s_vec(x_sb, dst, a_slice, tags):
                # one wide multiply + one segmented reduce over all C sub-rows
                scratch = work_pool.tile([P, C * F], fp32, tag=tags[0])
                s3 = scratch[:].rearrange("p (c f) -> p c f", c=C)
                nc.vector.tensor_tensor(
                    s3, x_sb[:].rearrange("p (c f) -> p c f", c=C),
                    a_slice.unsqueeze(1).broadcast_to((P, C, F)),
                    op=mybir.AluOpType.mult,
                )
                nc.vector.tensor_reduce(
                    dst[:].unsqueeze(2), s3,
                    axis=mybir.AxisListType.X, op=mybir.AluOpType.add,
                )

            def sdst_to_ccin(sd, psum_tag, sb_tag):
                tp = ps1_pool.tile([C, P], fp32, tag=psum_tag)
                nc.tensor.transpose(tp[:], sd[:], ident[:])
                tsd = const_pool.tile([C, P], fp32, tag=sb_tag)
                nc.vector.tensor_copy(tsd[:], tp[:])
                # cc_in bytes in local row order c*P+p ([C, P] view of the
                # [P, C] tile's flat storage)
                nc.sync.dma_start(
                    cc_in[:].rearrange("p c -> (p c)").rearrange(
                        "(a b) -> a b", a=C), tsd[:])
                return tsd

            if repeat > 1:
                # bench prologue: produce cc_out once (collectives cannot
                # sit inside a For_i), loop re-reads it every iteration.
                x0 = load_x("x0")
                sd0 = const_pool.tile([P, C], fp32, tag="sd0")
                s_vec(x0, sd0, ab_sb[:, F:], ("mv0a", "mv0b"))
                sdst_to_ccin(sd0, "tp0", "tsd0")
                collective()


            loop_cm = tc.For_i(0, repeat, 1) if repeat > 1 else nullcontext()
            with loop_cm:
                x_sb = load_x("x_sb")
                sd = const_pool.tile([P, C], fp32, tag="sd")
                s_vec(x_sb, sd, ab_sb[:, F:], ("mv1a", "mv1b"))
                tsd = sdst_to_ccin(sd, "tp1", "tsd1")
                if repeat == 1:
                    collective()
                s_src = const_pool.tile([P, C], fp32, tag="s_src")
                s_vec(x_sb, s_src, ab_sb[:, :F], ("mv2a", "mv2b"))

                # output row c*128+p at [partition p, sub-row c]
                out_view = out_dram.ap().rearrange("(c p) n -> p c n", c=C)

                g_sb = const_pool.tile([1, N], fp32, tag="g_sb")
                nc.sync.dma_start(
                    g_sb[:], cc_out[:].rearrange("a b -> (a b)").unsqueeze(0)
                )
                for k in range(N // NB2):
                    dbk = dbc_pool.tile([P, NB2], fp32, tag="dbk")
                    for h in range(NB2 // NB):
                        d_ps = psum_pool.tile([P, NB], fp32, tag="d_ps")
                        col = k * NB2 + h * NB
                        nc.tensor.matmul(
                            d_ps[:], ones_sb[:], g_sb[0:1, col:col + NB],
                            start=True, stop=True,
                        )
                        nc.vector.tensor_copy(dbk[:, h * NB:(h + 1) * NB], d_ps[:])
                    for t in range(C):
                        o = out_pool.tile([P, NB2], fp32, tag="o")
                        nc.scalar.activation(
                            o[:], dbk[:], mybir.ActivationFunctionType.Prelu,
                            bias=s_src[:, t:t + 1], scale=1.0, alpha=ALPHA,
                        )
                        nc.sync.dma_start(
                            out_view[:, t, k * NB2:(k + 1) * NB2], o[:]
                        )

    nc.compile()
    return nc


def _get_nc(repeat=1):
    if repeat not in _CACHE:
        _CACHE[repeat] = _build(repeat)
    return _CACHE[repeat]


def kernel(feature_matrix: np.ndarray, attention_vector: np.ndarray) -> np.ndarray:
    from concourse.bass_utils import run_bass_kernel_spmd

    feature_matrix = np.ascontiguousarray(feature_matrix, dtype=np.float32)
    attention_vector = np.ascontiguousarray(attention_vector, dtype=np.float32)

    nc = _get_nc()
    in_maps = [
        {
            "feature_matrix": feature_matrix[c * ROWS:(c + 1) * ROWS],
            "attention_vector": attention_vector,
        }
        for c in range(NCORES)
    ]
    res = run_bass_kernel_spmd(nc, in_maps, core_ids=list(range(NCORES)))
    return np.concatenate([res.results[c]["out"] for c in range(NCORES)], axis=0)


# revision 26
# speedup vs baseline: 1.1078x; 1.1078x over previous
"""Trainium2 Bass kernel for GAT-style attention score computation.

Math (see reference):
    s_src = X @ a[:F];  s_dst = X @ a[F:]
    e[i, j] = leaky_relu(s_src[i] + s_dst[j], alpha=0.2)

Sharding over 8 NeuronCores: row-shard X (1024 rows/core). Each core
computes its local s_src/s_dst slices, AllGathers s_dst (8192 floats),
and emits its [1024, 8192] row block of e.

Per-core dataflow (v3, "pipe"):
  - Row mapping: local row c*128 + p lives at partition p, sub-row c.
    Each [128, 8192] output store is then ONE contiguous 4 MB HBM block
    (measured 346 GB/s vs 317 GB/s for the interleaved mapping).
  - X shard loaded as one SBUF tile [128, 2048] via a 3-D AP.
  - a_src/a_dst broadcast to [128, 256] via a ones[1,128] matmul.
  - s_dst/s_src matvecs as per-sub-row DVE multiply+reduce pairs.
  - s_dst [128, 8] -> PE transpose (identity built on-chip with
    affine_select) -> [8, 128] -> DRAM in local row order -> AllGather.
  - Gathered s_dst [1, 8192] -> broadcast + activation pipelined in
    1024-column chunks: per chunk 2 ones-matmuls + copies -> dbk
    [128, 1024], then 8 ScalarE activations Lrelu(dbk + s_src[:, t])
    each DMA'd to a contiguous 512 KB block of the output.
    First output DMA starts ~5 us after the gather lands; the bcast
    build for chunk k+1 overlaps chunk k's stores.

The kernel is output-write bound: 32 MB/core of f32 at ~358 GB/s HBM
(~97 us floor); ScalarE activation total is ~73 us and hides under it.

`repeat` (bench-only): wraps the body in a For_i hardware loop with the
collective hoisted to a prologue (collectives inside a loop desync the
mesh), so device time can be measured by slope against the repeat count
(the axon per-call dispatch overhead is ~68 ms and swamps a single
execution).
"""

import numpy as np

N = 8192
F = 256
NCORES = 8
ROWS = N // NCORES          # 1024 rows per core
P = 128                     # partitions
C = ROWS // P               # 8 sub-rows per partition
ALPHA = 0.2
NB = 512                    # PSUM-bank-sized bcast matmul width
NB2 = 1024                  # column chunk width of the act/store pipeline

_CACHE = {}


def _build(repeat=1):
    import concourse.bacc as bacc
    import concourse.bass as bass
    import concourse.tile as tile
    from concourse import mybir
    from contextlib import nullcontext

    fp32 = mybir.dt.float32

    nc = bacc.Bacc(
        "TRN2",
        target_bir_lowering=False,
        debug=False,
        num_devices=NCORES,
    )

    x_dram = nc.dram_tensor("feature_matrix", [ROWS, F], fp32, kind="ExternalInput")
    av_dram = nc.dram_tensor("attention_vector", [2 * F, 1], fp32, kind="ExternalInput")
    out_dram = nc.dram_tensor("out", [ROWS, N], fp32, kind="ExternalOutput")

    with tile.TileContext(nc) as tc:
        with (
            tc.tile_pool(name="const", bufs=1) as const_pool,
            tc.tile_pool(name="work", bufs=2) as work_pool,
            tc.tile_pool(name="dbc", bufs=2) as dbc_pool,
            tc.tile_pool(name="outp", bufs=8) as out_pool,
            tc.tile_pool(name="psum", bufs=4, space=bass.MemorySpace.PSUM) as psum_pool,
            tc.tile_pool(name="psum1", bufs=1, space=bass.MemorySpace.PSUM) as ps1_pool,
            tc.tile_pool(name="dram", bufs=1, space="DRAM") as dram_pool,
        ):
            cc_in = dram_pool.tile([P, C], fp32, tag="cc_in")
            cc_out = dram_pool.tile([2 * C, N // (2 * C)], fp32, tag="cc_out")

            def collective():
                nc.gpsimd.collective_compute(
                    "AllGather",
                    mybir.AluOpType.bypass,
                    replica_groups=[list(range(NCORES))],
                    ins=[cc_in[:].opt()],
                    outs=[cc_out[:].opt()],
                )

            # ---- constants shared by prologue and loop ----
            av_sb = const_pool.tile([1, 2 * F], fp32, tag="av_sb")
            nc.sync.dma_start(av_sb[:], av_dram.ap().rearrange("f one -> one f"))
            ones_sb = const_pool.tile([1, P], fp32, tag="ones_sb")
            nc.vector.memset(ones_sb[:], 1.0)
            ident = const_pool.tile([P, P], fp32, tag="ident")
            nc.gpsimd.memset(ident[:], 1.0)
            nc.gpsimd.affine_select(
                ident[:], ident[:], pattern=[[1, P]],
                compare_op=mybir.AluOpType.is_equal, fill=0.0,
                base=0, channel_multiplier=-1,
            )
            a_ps = ps1_pool.tile([P, 2 * F], fp32, tag="a_ps")
            nc.tensor.matmul(a_ps[:], ones_sb[:], av_sb[:], start=True, stop=True)
            ab_sb = const_pool.tile([P, 2 * F], fp32, tag="ab_sb")
            nc.vector.tensor_copy(ab_sb[:], a_ps[:])

            def load_x(tag):
                x_sb = const_pool.tile([P, C * F], fp32, tag=tag)
                nc.sync.dma_start(
                    x_sb[:].rearrange("p (c f) -> p c f", c=C),
                    x_dram.ap().rearrange("(c p) f -> p c f", c=C),
                )
                return x_sb

            def s_vec(x_sb, dst, a_slice, tags):
                # per-sub-row multiply+reduce pairs pipeline on DVE (measured
                # faster than one wide [P, C*F] op pair)
                for c in range(C):
                    scratch = work_pool.tile([P, F], fp32, tag=tags[0])
                    nc.vector.tensor_tensor(
                        scratch[:], x_sb[:, c * F:(c + 1) * F], a_slice,
                        op=mybir.AluOpType.mult,
                    )
                    nc.vector.tensor_reduce(
                        dst[:, c:c + 1], scratch[:],
                        axis=mybir.AxisListType.X, op=mybir.AluOpType.add,
                    )

            def sdst_to_ccin(sd, psum_tag, sb_tag):
                tp = ps1_pool.tile([C, P], fp32, tag=psum_tag)
                nc.tensor.transpose(tp[:], sd[:], ident[:])
                tsd = const_pool.tile([C, P], fp32, tag=sb_tag)
                nc.vector.tensor_copy(tsd[:], tp[:])
                # cc_in bytes in local row order c*P+p ([C, P] view of the
                # [P, C] tile's flat storage)
                nc.sync.dma_start(
                    cc_in[:].rearrange("p c -> (p c)").rearrange(
                        "(a b) -> a b", a=C), tsd[:])
                return tsd

            if repeat > 1:
                # bench prologue: produce cc_out once (collectives cannot
                # sit inside a For_i), loop re-reads it every iteration.
                x0 = load_x("x0")
                sd0 = const_pool.tile([P, C], fp32, tag="sd0")
                s_vec(x0, sd0, ab_sb[:, F:], ("mv0a", "mv0b"))
                sdst_to_ccin(sd0, "tp0", "tsd0")
                collective()

            loop_cm = tc.For_i(0, repeat, 1) if repeat > 1 else nullcontext()
            with loop_cm:
                x_sb = load_x("x_sb")
                sd = const_pool.tile([P, C], fp32, tag="sd")
                s_vec(x_sb, sd, ab_sb[:, F:], ("mv1a", "mv1b"))
                sdst_to_ccin(sd, "tp1", "tsd1")
                if repeat == 1:
                    collective()
                s_src = const_pool.tile([P, C], fp32, tag="s_src")
                s_vec(x_sb, s_src, ab_sb[:, :F], ("mv2a", "mv2b"))

                # output row c*128+p at [partition p, sub-row c]
                out_view = out_dram.ap().rearrange("(c p) n -> p c n", c=C)

                g_sb = const_pool.tile([1, N], fp32, tag="g_sb")
                nc.sync.dma_start(
                    g_sb[:], cc_out[:].rearrange("a b -> (a b)").unsqueeze(0)
                )
                for k in range(N // NB2):
                    dbk = dbc_pool.tile([P, NB2], fp32, tag="dbk")
                    for h in range(NB2 // NB):
                        d_ps = psum_pool.tile([P, NB], fp32, tag="d_ps")
                        col = k * NB2 + h * NB
                        nc.tensor.matmul(
                            d_ps[:], ones_sb[:], g_sb[0:1, col:col + NB],
                            start=True, stop=True,
                        )
                        nc.vector.tensor_copy(dbk[:, h * NB:(h + 1) * NB], d_ps[:])
                    for t in range(C):
                        o = out_pool.tile([P, NB2], fp32, tag="o")
                        nc.scalar.activation(
                            o[:], dbk[:], mybir.ActivationFunctionType.Prelu,
                            bias=s_src[:, t:t + 1], scale=1.0, alpha=ALPHA,
                        )
                        nc.sync.dma_start(
                            out_view[:, t, k * NB2:(k + 1) * NB2], o[:]
                        )

    nc.compile()
    return nc


def _get_nc(repeat=1):
    if repeat not in _CACHE:
        _CACHE[repeat] = _build(repeat)
    return _CACHE[repeat]


def kernel(feature_matrix: np.ndarray, attention_vector: np.ndarray) -> np.ndarray:
    from concourse.bass_utils import run_bass_kernel_spmd

    feature_matrix = np.ascontiguousarray(feature_matrix, dtype=np.float32)
    attention_vector = np.ascontiguousarray(attention_vector, dtype=np.float32)

    nc = _get_nc()
    in_maps = [
        {
            "feature_matrix": feature_matrix[c * ROWS:(c + 1) * ROWS],
            "attention_vector": attention_vector,
        }
        for c in range(NCORES)
    ]
    res = run_bass_kernel_spmd(nc, in_maps, core_ids=list(range(NCORES)))
    return np.concatenate([res.results[c]["out"] for c in range(NCORES)], axis=0)
